# revision 1
# baseline (speedup 1.0000x reference)
"""Trainium2 Bass kernel for nn_ConfidenceDrivenMaskLayer3D.

Reference computation (per batch):
    k = gauss_65tap;  om = 1-mask;  init = om
    repeat 7: x = blur3d_separable(init); init = x*mask + om
    return last x*mask

Strategy (8 cores, zero inter-core communication):
  - Shard = (batch b in {0,1}) x (H-window c in {0..3}). Windows
    [0,128),[32,160),[96,224),[128,256) lie fully inside H=256, giving every
    core a [D=96, H'=128, W=256] sub-volume with >=32-voxel halo around its
    64-row core region. The Gaussian has pixel std 1.625, so 7 iterations
    propagate < 1e-10 of amplitude across 32 voxels -- halo truncation is
    far below fp32 noise. Host slices/assembles.
  - Each 1D conv pass is a banded-Toeplitz matmul on the TensorEngine in
    "stationary-data" form: lhsT = data tile, rhs = Toeplitz => output is
    conv'd along the old partition axis AND transposed into a new layout.
    Cycle per iteration: L_H --convH--> L_D --convD--> L_W --convW--> L_H.
    All three layouts live in ONE SBUF buffer (in-place, column-disjoint;
    Tile's access tracker serializes the overlapping columns).
  - Pointwise uses mask binarity: init = max(x, om), final out = min(x, m)
    (exact for m in {0,1} given x in [0,1]) -- fused into the PSUM
    evacuation of the last pass as a single tensor_tensor op.
"""
import json
import sys

sys.path.insert(0, "/opt/trn_rl_repo")

import numpy as np

SIZE = 65
SIGMA = 1.0 / 40.0
ITERS = 7
D, HP, W = 96, 128, 256
H_FULL = 256
WINDOWS = [(0, 128), (32, 160), (96, 224), (128, 256)]

# ---------------------------------------------------------------------------
# walrus workaround: this container's walrus rejects instructions carrying
# more than one sync wait ("Too many sync wait commands"). Split extra waits
# into single-wait NoOps on the same engine (FIFO order preserves semantics).
# ---------------------------------------------------------------------------
_waitfix_done = False


def _split_waits_in_bir(bir):
    n = 0
    for fn in bir.get("functions", []):
        for blk in fn.get("blocks", []):
            new = []
            for inst in blk.get("instructions", []):
                si = inst.get("sync_info")
                ow = (si or {}).get("on_wait") or []
                if len(ow) > 1 and inst.get("engine") not in (None, "Unassigned"):
                    for j, w in enumerate(ow[:-1]):
                        new.append({
                            "debug": inst.get("debug", 0),
                            "engine": inst["engine"],
                            "ins": [], "outs": [],
                            "name": f"{inst['name']}-wsplit{j}",
                            "opcode": "NoOp",
                            "sync_info": {"on_update": [], "on_wait": [w]},
                        })
                        n += 1
                    si["on_wait"] = [ow[-1]]
                new.append(inst)
            blk["instructions"] = new
    return n


def _install_waitfix():
    global _waitfix_done
    if _waitfix_done:
        return
    _waitfix_done = True
    import concourse.bass2jax as b2j
    import concourse.bass_utils as bu

    orig = bu.compile_bir_kernel

    def patched(bir_bytes, *a, **k):
        bir = json.loads(bir_bytes)
        _split_waits_in_bir(bir)
        return orig(json.dumps(bir).encode(), *a, **k)

    bu.compile_bir_kernel = patched
    b2j.compile_bir_kernel = patched


# ---------------------------------------------------------------------------
# host-side constants
# ---------------------------------------------------------------------------
def gauss_k1d():
    x = (np.arange(SIZE, dtype=np.float64) - (SIZE - 1) / 2.0) / SIZE
    k = np.exp(-(x * x) / (2.0 * SIGMA * SIGMA))
    return (k / k.sum()).astype(np.float32)


def toeplitz(n, k1d):
    i = np.arange(n)[:, None]
    j = np.arange(n)[None, :]
    t = i - j + (SIZE - 1) // 2
    return np.where((t >= 0) & (t < SIZE),
                    k1d[np.clip(t, 0, SIZE - 1)], 0.0).astype(np.float32)


# ---------------------------------------------------------------------------
# device kernel builder (parameterized so small configs can be tested)
# ---------------------------------------------------------------------------
def build_kernel(d=D, hp=HP, w=W, iters=ITERS, repeats=1):
    """All cross-group orderings flow through RAW dependency chains only
    (Tile's WAR tracking across overlapping strided views proved unsound):
      - passes 1+2 run per 128-wide W-half, bouncing L_D through a separate
        C buffer;
      - L_W uses an interleaved free layout (d, wb, h) occupying exactly the
        same bytes as L_H's (d, w): pass 2's writes land in the half of L_H
        already fully consumed (transitively via C's RAW chain), and pass 3's
        in-place writes are exactly group-aligned (write range g == its own
        matmuls' read range g; disjoint across groups).
    Layouts (B free index per partition, fp32 elems):
      L_H:  voxel(d,h,w)  at [part h,  d*w + w]
      L_W2: voxel(d,h,w)  at [part w%128,  d*w + (w//128)*128 + h]
      C:    voxel(d,h,w)  at [part d,  h*128 + (w%128)]  (per W-half)
    """
    import concourse.bass as bass
    import concourse.mybir as mybir
    import concourse.tile as tile

    F32 = mybir.dt.float32
    WB = w // 128
    assert w % 128 == 0 and hp <= 128 and d <= 128
    assert d % 2 == 0 and hp % 4 == 0 and (w // 4) % 1 == 0

    nc = bass.Bass()
    om_lh = nc.declare_dram_parameter("om_lh", [hp, d, w], F32, isOutput=False)
    m_lh = nc.declare_dram_parameter("m_lh", [hp, d, w], F32, isOutput=False)
    toep_h = nc.declare_dram_parameter("toep_h", [hp, hp], F32, isOutput=False)
    toep_d = nc.declare_dram_parameter("toep_d", [d, d], F32, isOutput=False)
    toep_w = nc.declare_dram_parameter("toep_w", [128, WB, w], F32, isOutput=False)
    out_t = nc.declare_dram_parameter("out", [hp, d, w], F32, isOutput=True)

    OMCH = 4                      # d slices per streamed om/m chunk
    assert d % OMCH == 0

    with tile.TileContext(nc) as tc:
        with (
            tc.tile_pool(name="state", bufs=1) as state_pool,
            tc.tile_pool(name="singles", bufs=1) as singles,
            tc.tile_pool(name="omstream", bufs=2) as omp,
            tc.tile_pool(name="psum", bufs=6, space="PSUM") as psum,
        ):
            B = state_pool.tile([128, d * w], F32)
            C = state_pool.tile([d, hp * 128], F32, tag="cbuf")
            B_lh = B[0:hp, :].rearrange("p (d w) -> p d w", d=d)
            # interleaved L_W: [part wlo, (d, wb, h)]
            B_lw = B[:, :].rearrange("p (d b h) -> p d b h", d=d, b=WB)
            C_ld = C[:, :].rearrange("p (h w) -> p h w", h=hp)

            th = singles.tile([hp, hp], F32)
            td = singles.tile([d, d], F32)
            tw = singles.tile([128, WB, w], F32)
            nc.sync.dma_start(out=th[:], in_=toep_h[:])
            nc.sync.dma_start(out=td[:], in_=toep_d[:])
            nc.sync.dma_start(out=tw[:], in_=toep_w[:])

            for rep in range(repeats):
                # initial state: init_0 = om (reload each repeat)
                nc.sync.dma_start(
                    out=B[0:hp, :],
                    in_=om_lh[:].rearrange("p d w -> p (d w)"),
                )

                for it in range(iters):
                    last = it == iters - 1

                    for wh in range(WB):
                        # -- pass 1 (W-half wh): conv along H; L_H -> C --
                        for g in range(32):
                            pt = psum.tile([d, 4, hp], F32, tag="ps")
                            for j in range(4):
                                wloc = g * 4 + j
                                nc.tensor.matmul(
                                    pt[:, j, :],
                                    B_lh[:, :, wh * 128 + wloc],
                                    th[:], start=True, stop=True)
                            dest = C_ld[:, :, g * 4:(g + 1) * 4].rearrange(
                                "p h w -> p w h")
                            if g % 4 == 0:
                                nc.vector.tensor_copy(dest, pt[:])
                            else:
                                nc.scalar.copy(dest, pt[:])

                        # -- pass 2 (W-half wh): conv along D; C -> L_W2 --
                        for g in range(hp // 4):
                            pt = psum.tile([128, 4, d], F32, tag="ps")
                            for jh in range(4):
                                nc.tensor.matmul(
                                    pt[:, jh, :],
                                    C_ld[:, g * 4 + jh, :],
                                    td[:], start=True, stop=True)
                            dest = B_lw[:, :, wh, g * 4:(g + 1) * 4].rearrange(
                                "p d h -> p h d")
                            if g % 4 == 0:
                                nc.vector.tensor_copy(dest, pt[:])
                            else:
                                nc.scalar.copy(dest, pt[:])

                    # ---- pass 3: conv along W + fused pointwise; L_W2 -> L_H ----
                    stream_src = m_lh if last else om_lh
                    omc = None
                    for g in range(d // 2):
                        d0 = g * 2
                        if d0 % OMCH == 0:
                            omc = omp.tile([hp, OMCH, w], F32, tag="om")
                            nc.sync.dma_start(
                                out=omc[:],
                                in_=stream_src[:, d0:d0 + OMCH, :])
                        pt = psum.tile([hp, 2, w], F32, tag="ps")
                        for jd in range(2):
                            for wb in range(WB):
                                nc.tensor.matmul(
                                    pt[:, jd, :], B_lw[:, d0 + jd, wb, 0:hp],
                                    tw[:, wb, :],
                                    start=(wb == 0), stop=(wb == WB - 1))
                        nc.vector.tensor_tensor(
                            out=B_lh[:, d0:d0 + 2, :],
                            in0=pt[:],
                            in1=omc[:, d0 % OMCH:d0 % OMCH + 2, :],
                            op=(mybir.AluOpType.min if last
                                else mybir.AluOpType.max))

                # write result (full local volume; host slices core rows)
                nc.sync.dma_start(
                    out=out_t[:].rearrange("p d w -> p (d w)"),
                    in_=B[0:hp, :])
    return nc


# ---------------------------------------------------------------------------
# host wrapper
# ---------------------------------------------------------------------------
_cached = {}


def _get_kernel(repeats=1):
    key = repeats
    if key not in _cached:
        _cached[key] = build_kernel(repeats=repeats)
    return _cached[key]


def make_in_maps(mask):
    """mask: [2,1,96,256,256] float32 -> list of 8 per-core input dicts."""
    k1d = gauss_k1d()
    tws = toeplitz(W, k1d)
    toep = {
        "toep_h": toeplitz(HP, k1d),
        "toep_d": toeplitz(D, k1d),
        "toep_w": np.ascontiguousarray(
            tws.reshape(2, 128, W).transpose(1, 0, 2)),  # [p, wb, w]
    }
    in_maps = []
    for b in range(mask.shape[0]):
        for c, (lo, hi) in enumerate(WINDOWS):
            m_lh = np.ascontiguousarray(
                mask[b, 0, :, lo:hi, :].transpose(1, 0, 2))  # [h, d, w]
            in_maps.append({
                "m_lh": m_lh,
                "om_lh": np.ascontiguousarray(1.0 - m_lh),
                **toep,
            })
    return in_maps


def assemble(results, mask_shape):
    out = np.zeros(mask_shape, np.float32)
    i = 0
    for b in range(mask_shape[0]):
        for c, (lo, hi) in enumerate(WINDOWS):
            res = results[i]["out"]          # [h=128, d, w]
            c0 = 64 * c - lo
            out[b, 0, :, 64 * c:64 * c + 64, :] = \
                res[c0:c0 + 64, :, :].transpose(1, 0, 2)
            i += 1
    return out


def kernel(mask):
    mask = np.ascontiguousarray(np.asarray(mask), dtype=np.float32)
    _install_waitfix()
    from concourse.bass_utils import run_bass_kernel_spmd

    nc = _get_kernel()
    in_maps = make_in_maps(mask)
    res = run_bass_kernel_spmd(nc, in_maps, list(range(8))).results
    return assemble(res, mask.shape)


if __name__ == "__main__":
    rng = np.random.default_rng(0)
    m = (rng.random((2, 1, D, H_FULL, W)) > 0.5).astype(np.float32)
    out = kernel(m)
    print("ran; out stats:", out.min(), out.max(), out.mean())

